# revision 18
# baseline (speedup 1.0000x reference)
"""Trainium2 Bass kernel for nn_CANN_75857712382071.

Single-head self-attention (B=32, A=2048, D=128) with scalar output
projection, algebraically collapsed:

    out[b,aq] = (sum_ak E * (w+c+bo)) / (sum_ak E)
    E = exp(scale * (z M z^T + 1 (x) g)),  M = Wq^T Wk
    g[ak] = z[ak] . (Wk^T bq),   w[ak] = z[ak] . (Wv^T Wo^T)

q/k/v/h are never materialized; softmax max-subtraction is skipped
(logits are O(10); softmax is shift-invariant in exact arithmetic).

Data-parallel over batch: 4 batches per core on 8 NeuronCores.

Engine budget per batch (measured): PE 33us (128 512-wide matmuls:
scores + num/den reduction), ScalarE 36us if it does all 32 exps.
To balance, 4 of 32 exp tiles per batch go to DVE using a Schraudolph
bf16 bit-trick (int16(A*s+B) bitcast as bf16 ~= exp(scale*s), C
calibrated for truncation); DVE reads PSUM directly (GpSimd cannot).

Schedule highlights:
  - aq-half-major loop: nd accumulators need 1-2 PSUM banks -> 3
    rotating [128,1024] score slots + dedicated UT-quarter bank.
  - zT via 2 DRAM-direct xbar-DMA transposes per batch (z uploaded
    bf16); zn staged separately only for the w column (DVE).
  - bulk DMA on the sync HWDGE queue (16-engine fan-out); nothing on
    ScalarE's queue.
  - single combined output DMA at the end (4KB teardown-cheap).
  - batch b+1 setup interleaved into b's loop; finale of b-1 inside b.
"""

import sys
import types

import numpy as np

N_CORES = 8
B, A, D = 32, 2048, 128
B_PER = B // N_CORES
SCALE = float(D) ** -0.5

SCHR_TKS = (3, 7, 11, 15)     # exp tiles handled by DVE bit-trick
SCHR_C = 6.0
ND1BANK = True                # nd chunks packed into one PSUM bank


def _install_axon_shim():
    """Allow run_bass_kernel_spmd(trace=True) to NTFF-profile under axon."""
    try:
        import antenv  # noqa: F401
    except ImportError:
        return
    if "antenv.axon_hooks" not in sys.modules:
        mod = types.ModuleType("antenv.axon_hooks")
        _hook = [None]
        mod.set_axon_ntff_profile_hook = lambda h: _hook.__setitem__(0, h)
        mod.get_axon_ntff_profile_hook = lambda: _hook[0]
        sys.modules["antenv.axon_hooks"] = mod
    from antenv.axon_hooks import (
        get_axon_ntff_profile_hook,
        set_axon_ntff_profile_hook,
    )
    if get_axon_ntff_profile_hook() is None:
        try:
            from trn_agent_boot.trn_boot import _ntff_profile_via_ctypes
            set_axon_ntff_profile_hook(
                _ntff_profile_via_ctypes("/opt/axon/libaxon_pjrt.so"))
        except Exception:
            pass
    try:
        from concourse import bass_utils
        bass_utils.upload_artifacts = lambda tmpdir: tmpdir
    except Exception:
        pass


def _build_program(cbo: float):
    import concourse.bacc as bacc
    import concourse.mybir as mybir
    import concourse.tile as tile

    f32 = mybir.dt.float32
    bf16 = mybir.dt.bfloat16
    i16 = mybir.dt.int16
    AF = mybir.ActivationFunctionType
    ADD = mybir.AluOpType.add
    MULT = mybir.AluOpType.mult

    A16S = float(128.0 / np.log(2.0) * SCALE)
    B16S = float(127.0 * 128.0 - SCHR_C)

    nc = bacc.Bacc("TRN2", target_bir_lowering=False, debug=False,
                   num_devices=N_CORES, num_swdge_queues=2)

    z_d = nc.dram_tensor("z", [B_PER, A, D], bf16, kind="ExternalInput").ap()
    m_d = nc.dram_tensor("m_lhs", [D, D], f32, kind="ExternalInput").ap()
    gw_d = nc.dram_tensor("gw", [D, 1], f32, kind="ExternalInput").ap()
    wvb_d = nc.dram_tensor("wvb", [128, A], bf16, kind="ExternalInput").ap()
    out_d = nc.dram_tensor("out", [B_PER, A], f32, kind="ExternalOutput").ap()

    NT = A // 128          # 16 key tiles
    NH = 2                 # aq halves (1024 each)

    with tile.TileContext(nc) as tc:
        with (
            tc.tile_pool(name="sb", bufs=1) as sb,
            tc.tile_pool(name="ps", bufs=3, space="PSUM") as ps,
        ):
            # ---- constants ----
            m_f = sb.tile([D, D], f32)
            nc.sync.dma_start(m_f[:], m_d[:])
            gw_col = sb.tile([D, 1], f32)
            nc.sync.dma_start(gw_col[:], gw_d[:])
            m_r = sb.tile([D, D], bf16)
            nc.vector.tensor_copy(m_r[:], m_f[:])
            wvb = sb.tile([128, A], bf16)

            # ACT table warmup (overlaps first z DMAs)
            warm = sb.tile([D, 1], f32)
            nc.scalar.activation(warm[:], gw_col[:], AF.Exp, scale=0.0)

            # PE HAM pre-warm on junk data while the first DMAs fly
            junk = sb.tile([128, 512], bf16)
            nc.gpsimd.memset(junk[:], 0.0)
            pjunk = ps.tile([128, 512], f32, name="pjunk", tag="sc")
            for i in range(28):
                nc.tensor.matmul(pjunk[:, 0:128], junk[:, 0:128],
                                 junk[:, 0:128], start=True, stop=True)

            st = {}  # per-batch live tiles

            def emit_transpose(b, h):
                # DRAM-direct xbar transpose of one aq half into zT
                s = st.setdefault(b, {})
                if h == 0:
                    s["zT"] = sb.tile([D, A], bf16, name=f"zT{b}", tag="zT",
                                      bufs=2)
                nc.sync.dma_start_transpose(
                    out=s["zT"][:, h * 1024:(h + 1) * 1024],
                    in_=z_d[b][h * 1024:(h + 1) * 1024, :])

            def emit_z_dma(b, grp):
                # zn staging (w column only): 2 fat chunks on sync HWDGE
                s = st[b]
                if grp == 0:
                    s["zn"] = sb.tile([128, A], bf16, name=f"zn{b}",
                                      tag="zn", bufs=2)
                zsrc = z_d[b].rearrange("(t p) d -> p t d", p=128)
                zdst = s["zn"].rearrange("p (t d) -> p t d", d=D)
                nc.sync.dma_start(zdst[:, 8 * grp:8 * grp + 8],
                                  zsrc[:, 8 * grp:8 * grp + 8])

            def emit_w_mult(b):
                s = st[b]
                s["scr"] = sb.tile([128, A], bf16, name=f"scr{b}",
                                   tag="scr", bufs=2)
                nc.vector.tensor_tensor(s["scr"][:], s["zn"][:], wvb[:],
                                        MULT)

            def emit_w_reduce(b):
                s = st[b]
                s["wacc"] = sb.tile([128, NT], f32, name=f"wacc{b}",
                                    tag="wacc", bufs=2)
                scr3 = s["scr"].rearrange("p (t d) -> p t d", d=D)
                nc.vector.tensor_reduce(
                    s["wacc"][:], scr3[:], axis=mybir.AxisListType.X,
                    op=ADD)

            def emit_wl(b):
                s = st[b]
                wl = sb.tile([128, 2 * NT], bf16, name=f"wl{b}", tag="wl",
                             bufs=2)
                nc.gpsimd.memset(wl[:], 1.0)
                wl3 = wl.rearrange("p (t two) -> p t two", two=2)
                nc.vector.tensor_scalar(wl3[:, :, 0], s["wacc"][:], cbo,
                                        None, ADD)
                s["wl"] = wl

            def emit_ut_quarter(b, q):
                s = st[b]
                if q == 0:
                    s["UT"] = sb.tile([D, A], bf16, name=f"UT{b}", tag="UT",
                                      bufs=2)
                pu = ps.tile([128, 512], f32, name=f"pu{b}_{q}", tag="pu",
                             bufs=1)
                o = q * 512
                nc.tensor.matmul(pu[:], m_r[:], s["zT"][:, o:o + 512],
                                 start=True, stop=True)
                nc.scalar.activation(s["UT"][:, o:o + 512], pu[:],
                                     AF.Identity, bias=gw_col[:],
                                     scale=1.0)

            def emit_scores(b, h, tk):
                s = st[b]
                lhs = s["zT"][:, tk * 128:(tk + 1) * 128]
                ps_t = ps.tile([128, 1024], f32, name=f"s{b}_{h}_{tk}",
                               tag="sc")
                for j in range(2):
                    o = h * 1024 + j * 512
                    nc.tensor.matmul(ps_t[:, j * 512:(j + 1) * 512],
                                     lhs, s["UT"][:, o:o + 512],
                                     start=True, stop=True)
                eT = sb.tile([128, 1024], bf16, name=f"e{b}_{h}_{tk}",
                             tag="eT", bufs=10)
                if tk in SCHR_TKS:
                    nc.vector.tensor_scalar(eT.bitcast(i16)[:], ps_t[:],
                                            A16S, B16S, MULT, ADD)
                else:
                    nc.scalar.activation(eT[:], ps_t[:], AF.Exp,
                                         scale=SCALE)
                return eT

            def emit_nd(b, h, tk, eT):
                s = st[b]
                wlt = s["wl"][:, 2 * tk:2 * tk + 2]
                for c in range(2):
                    nc.tensor.matmul(
                        s["nd"][c], wlt, eT[:, c * 512:(c + 1) * 512],
                        start=(tk == 0), stop=(tk == NT - 1))

            def alloc_nd(b, h):
                s = st[b]
                if ND1BANK:
                    ndt = ps.tile([34, 512], f32, name=f"nd{b}_{h}",
                                  tag="nd", bufs=1)
                    s["nd"] = [ndt[0:2, :], ndt[32:34, :]]
                else:
                    s["nd"] = [ps.tile([2, 512], f32, name=f"nd{b}_{h}_{c}",
                                       tag=f"nd{c}", bufs=1)[:]
                               for c in range(2)]

            def emit_nd_flush(b, h):
                s = st[b]
                for c in range(2):
                    o = h * 1024 + c * 512
                    nc.vector.tensor_copy(s["ndall"][0:2, o:o + 512],
                                          s["nd"][c])

            o16 = sb.tile([16, B_PER * 128], f32, name="o16")

            def emit_finale(b, step):
                s = st[b]
                if step == 0:
                    s["num16"] = sb.tile([16, 128], f32, name=f"num16{b}",
                                         tag="num16", bufs=2)
                    nc.sync.dma_start(
                        s["num16"][:, :],
                        s["ndall"][0:1, :].rearrange(
                            "one (t p) -> one t p", p=128))
                elif step == 1:
                    s["den16"] = sb.tile([16, 128], f32, name=f"den16{b}",
                                         tag="den16", bufs=2)
                    nc.sync.dma_start(
                        s["den16"][:, :],
                        s["ndall"][1:2, :].rearrange(
                            "one (t p) -> one t p", p=128))
                elif step == 2:
                    s["rcp"] = sb.tile([16, 128], f32, name=f"rcp{b}",
                                       tag="rcp", bufs=2)
                    nc.vector.reciprocal(s["rcp"][:], s["den16"][:])
                else:
                    nc.vector.tensor_tensor(
                        o16[:, b * 128:(b + 1) * 128], s["num16"][:],
                        s["rcp"][:], MULT)
                    st.pop(b)

            # ---- prologue: batch 0 setup ----
            emit_transpose(0, 0)
            emit_transpose(0, 1)
            emit_z_dma(0, 0)
            emit_z_dma(0, 1)
            nc.sync.dma_start(wvb[:], wvb_d[:])
            for q in range(4):
                emit_ut_quarter(0, q)
            emit_w_mult(0)
            emit_w_reduce(0)
            emit_wl(0)

            def setup_piece(b, nxt, h, tk):
                # finale of b-1 early in b; setup of b+1 spread through b
                if h == 0 and tk in (1, 2, 3, 4) and (b - 1) in st:
                    emit_finale(b - 1, tk - 1)
                if nxt is None:
                    return
                # batch 0 is short (no fill phase): run its successor's
                # setup ~6 ticks earlier so DVE finishes UT(1) in time
                step = h * NT + tk
                if step == 5:
                    emit_transpose(nxt, 0)
                elif step == 7:
                    emit_transpose(nxt, 1)
                elif step == 9:
                    emit_z_dma(nxt, 0)
                elif step == 11:
                    emit_z_dma(nxt, 1)
                elif step == 16:
                    emit_w_mult(nxt)
                elif step == 18:
                    emit_w_reduce(nxt)
                elif step == 20:
                    emit_wl(nxt)
                elif step in (22, 24, 26, 28):
                    emit_ut_quarter(nxt, (step - 22) // 2)

            def emit_row_finale(b, h, part):
                # fast 1-descriptor path for the exposed last-batch tail;
                # part 0 (DMA) emitted well before part 1 (compute) so the
                # DVE never head-of-line blocks on a fresh DMA
                s = st[b]
                o = h * 1024
                if part == 0:
                    s[f"denr{h}"] = sb.tile([1, 1024], f32,
                                            name=f"denr{b}_{h}",
                                            tag=f"denr{h}", bufs=1)
                    nc.sync.dma_start(s[f"denr{h}"][:],
                                      s["ndall"][1:2, o:o + 1024])
                    return
                rcp_row = sb.tile([1, 1024], f32, name=f"rcpr{b}_{h}",
                                  tag="rcpr", bufs=2)
                nc.vector.reciprocal(rcp_row[:], s[f"denr{h}"][:])
                nc.vector.tensor_tensor(orow3[0:1, o:o + 1024],
                                        s["ndall"][0:1, o:o + 1024],
                                        rcp_row[:], MULT)

            orow3 = sb.tile([1, A], f32, name="orow3")

            for b in range(B_PER):
                s = st[b]
                s["ndall"] = sb.tile([2, A], f32, name=f"ndall{b}",
                                     tag="ndall", bufs=2)
                nxt = b + 1 if b + 1 < B_PER else None
                for h in range(NH):
                    alloc_nd(b, h)
                    pend = []
                    for tk in range(NT):
                        eT = emit_scores(b, h, tk)
                        pend.append((tk, eT))
                        keep = 3 if tk < 13 else 15 - tk
                        while len(pend) > keep:
                            ptk, peT = pend.pop(0)
                            emit_nd(b, h, ptk, peT)
                        setup_piece(b, nxt, h, tk)
                    emit_nd_flush(b, h)
                if nxt is None:
                    for stp in range(4):
                        emit_finale(b, stp)

            # single combined output DMA
            nc.sync.dma_start(
                out_d[:, :].rearrange("b (t p) -> t b p", p=128),
                o16.rearrange("t (b p) -> t b p", p=128))

    nc.compile()
    return nc


def run(inputs: dict, trace: bool = False):
    _install_axon_shim()
    import ml_dtypes
    from concourse.bass_utils import run_bass_kernel_spmd

    z = np.asarray(inputs["z"], dtype=np.float32)
    Wq = np.asarray(inputs["Wq"], dtype=np.float64)
    bq = np.asarray(inputs["bq"], dtype=np.float64)
    Wk = np.asarray(inputs["Wk"], dtype=np.float64)
    Wv = np.asarray(inputs["Wv"], dtype=np.float64)
    bv = np.asarray(inputs["bv"], dtype=np.float64)
    Wo = np.asarray(inputs["Wo"], dtype=np.float64)
    bo = np.asarray(inputs["bo"], dtype=np.float64)

    # host-side weight algebra (tiny, exact in float64)
    m_lhs = (Wq.T @ Wk).astype(np.float32)            # [d, d']
    gw = (Wk.T @ bq).astype(np.float32).reshape(D, 1)
    wv = (Wv.T @ Wo[0]).astype(np.float32)            # [d]
    wvb = np.broadcast_to(np.tile(wv, A // D), (128, A)).astype(
        ml_dtypes.bfloat16)
    cbo_val = float(bv @ Wo[0] + bo[0])

    z_bf = z.astype(ml_dtypes.bfloat16)

    nc = _build_program(cbo_val)

    in_maps = []
    for c in range(N_CORES):
        in_maps.append({
            "z": z_bf[c * B_PER:(c + 1) * B_PER],
            "m_lhs": m_lhs,
            "gw": gw,
            "wvb": np.ascontiguousarray(wvb),
        })
    res = run_bass_kernel_spmd(nc, in_maps, core_ids=list(range(N_CORES)),
                               trace=trace)
    out = np.concatenate([res.results[c]["out"] for c in range(N_CORES)],
                         axis=0)
    return out.reshape(B, A, 1).astype(np.float32), res


def kernel(**inputs) -> np.ndarray:
    out, _ = run(inputs, trace=False)
    return out


# revision 19
# speedup vs baseline: 1.0240x; 1.0240x over previous
"""Trainium2 Bass kernel for nn_CANN_75857712382071.

Single-head self-attention (B=32, A=2048, D=128) with scalar output
projection, algebraically collapsed:

    out[b,aq] = (sum_ak E * (w+c+bo)) / (sum_ak E)
    E = exp(scale * (z M z^T + 1 (x) g)),  M = Wq^T Wk
    g[ak] = z[ak] . (Wk^T bq),   w[ak] = z[ak] . (Wv^T Wo^T)

q/k/v/h are never materialized; softmax max-subtraction is skipped
(logits are O(10); softmax is shift-invariant in exact arithmetic).

Data-parallel over batch: 4 batches per core on 8 NeuronCores.

Engine budget per batch (measured): PE 33us (128 512-wide matmuls:
scores + num/den reduction), ScalarE 36us if it does all 32 exps.
To balance, 4 of 32 exp tiles per batch go to DVE using a Schraudolph
bf16 bit-trick (int16(A*s+B) bitcast as bf16 ~= exp(scale*s), C
calibrated for truncation); DVE reads PSUM directly (GpSimd cannot).

Schedule highlights:
  - aq-half-major loop: nd accumulators need 1-2 PSUM banks -> 3
    rotating [128,1024] score slots + dedicated UT-quarter bank.
  - zT via 2 DRAM-direct xbar-DMA transposes per batch (z uploaded
    bf16); zn staged separately only for the w column (DVE).
  - bulk DMA on the sync HWDGE queue (16-engine fan-out); nothing on
    ScalarE's queue.
  - single combined output DMA at the end (4KB teardown-cheap).
  - batch b+1 setup interleaved into b's loop; finale of b-1 inside b.
"""

import sys
import types

import numpy as np

N_CORES = 8
B, A, D = 32, 2048, 128
B_PER = B // N_CORES
SCALE = float(D) ** -0.5

SCHR_TKS = (3, 7, 11, 15)     # exp tiles handled by DVE bit-trick
SCHR_C = 6.0
ND1BANK = True                # nd chunks packed into one PSUM bank


def _install_axon_shim():
    """Allow run_bass_kernel_spmd(trace=True) to NTFF-profile under axon."""
    try:
        import antenv  # noqa: F401
    except ImportError:
        return
    if "antenv.axon_hooks" not in sys.modules:
        mod = types.ModuleType("antenv.axon_hooks")
        _hook = [None]
        mod.set_axon_ntff_profile_hook = lambda h: _hook.__setitem__(0, h)
        mod.get_axon_ntff_profile_hook = lambda: _hook[0]
        sys.modules["antenv.axon_hooks"] = mod
    from antenv.axon_hooks import (
        get_axon_ntff_profile_hook,
        set_axon_ntff_profile_hook,
    )
    if get_axon_ntff_profile_hook() is None:
        try:
            from trn_agent_boot.trn_boot import _ntff_profile_via_ctypes
            set_axon_ntff_profile_hook(
                _ntff_profile_via_ctypes("/opt/axon/libaxon_pjrt.so"))
        except Exception:
            pass
    try:
        from concourse import bass_utils
        bass_utils.upload_artifacts = lambda tmpdir: tmpdir
    except Exception:
        pass


def _build_program(cbo: float):
    import concourse.bacc as bacc
    import concourse.mybir as mybir
    import concourse.tile as tile

    f32 = mybir.dt.float32
    bf16 = mybir.dt.bfloat16
    i16 = mybir.dt.int16
    AF = mybir.ActivationFunctionType
    ADD = mybir.AluOpType.add
    MULT = mybir.AluOpType.mult

    A16S = float(128.0 / np.log(2.0) * SCALE)
    B16S = float(127.0 * 128.0 - SCHR_C)

    nc = bacc.Bacc("TRN2", target_bir_lowering=False, debug=False,
                   num_devices=N_CORES, num_swdge_queues=2)

    z_d = nc.dram_tensor("z", [B_PER, A, D], bf16, kind="ExternalInput").ap()
    m_d = nc.dram_tensor("m_lhs", [D, D], f32, kind="ExternalInput").ap()
    gw_d = nc.dram_tensor("gw", [D, 1], f32, kind="ExternalInput").ap()
    wvb_d = nc.dram_tensor("wvb", [128, A], bf16, kind="ExternalInput").ap()
    out_d = nc.dram_tensor("out", [B_PER, A], f32, kind="ExternalOutput").ap()

    NT = A // 128          # 16 key tiles
    NH = 2                 # aq halves (1024 each)

    with tile.TileContext(nc) as tc:
        with (
            tc.tile_pool(name="sb", bufs=1) as sb,
            tc.tile_pool(name="ps", bufs=3, space="PSUM") as ps,
        ):
            # ---- constants ----
            m_f = sb.tile([D, D], f32)
            nc.sync.dma_start(m_f[:], m_d[:])
            gw_col = sb.tile([D, 1], f32)
            nc.sync.dma_start(gw_col[:], gw_d[:])
            m_r = sb.tile([D, D], bf16)
            nc.vector.tensor_copy(m_r[:], m_f[:])
            wvb = sb.tile([128, A], bf16)

            # ACT table warmup (overlaps first z DMAs)
            warm = sb.tile([D, 1], f32)
            nc.scalar.activation(warm[:], gw_col[:], AF.Exp, scale=0.0)

            # PE HAM pre-warm on junk data while the first DMAs fly
            junk = sb.tile([128, 512], bf16)
            nc.gpsimd.memset(junk[:], 0.0)
            pjunk = ps.tile([128, 512], f32, name="pjunk", tag="sc")
            for i in range(28):
                nc.tensor.matmul(pjunk[:, 0:128], junk[:, 0:128],
                                 junk[:, 0:128], start=True, stop=True)

            st = {}  # per-batch live tiles

            def emit_transpose(b, h):
                # DRAM-direct xbar transpose of one aq half into zT
                s = st.setdefault(b, {})
                if h == 0:
                    s["zT"] = sb.tile([D, A], bf16, name=f"zT{b}", tag="zT",
                                      bufs=2)
                nc.sync.dma_start_transpose(
                    out=s["zT"][:, h * 1024:(h + 1) * 1024],
                    in_=z_d[b][h * 1024:(h + 1) * 1024, :])

            def emit_z_dma(b, grp):
                # zn staging (w column only): 2 fat chunks on sync HWDGE
                s = st[b]
                if grp == 0:
                    s["zn"] = sb.tile([128, A], bf16, name=f"zn{b}",
                                      tag="zn", bufs=2)
                zsrc = z_d[b].rearrange("(t p) d -> p t d", p=128)
                zdst = s["zn"].rearrange("p (t d) -> p t d", d=D)
                nc.sync.dma_start(zdst[:, 8 * grp:8 * grp + 8],
                                  zsrc[:, 8 * grp:8 * grp + 8])

            def emit_w_mult(b):
                s = st[b]
                s["scr"] = sb.tile([128, A], bf16, name=f"scr{b}",
                                   tag="scr", bufs=2)
                nc.vector.tensor_tensor(s["scr"][:], s["zn"][:], wvb[:],
                                        MULT)

            def emit_w_reduce(b):
                s = st[b]
                s["wacc"] = sb.tile([128, NT], f32, name=f"wacc{b}",
                                    tag="wacc", bufs=2)
                scr3 = s["scr"].rearrange("p (t d) -> p t d", d=D)
                nc.vector.tensor_reduce(
                    s["wacc"][:], scr3[:], axis=mybir.AxisListType.X,
                    op=ADD)

            def emit_wl(b):
                s = st[b]
                wl = sb.tile([128, 2 * NT], bf16, name=f"wl{b}", tag="wl",
                             bufs=2)
                nc.gpsimd.memset(wl[:], 1.0)
                wl3 = wl.rearrange("p (t two) -> p t two", two=2)
                nc.vector.tensor_scalar(wl3[:, :, 0], s["wacc"][:], cbo,
                                        None, ADD)
                s["wl"] = wl

            def emit_ut_quarter(b, q):
                s = st[b]
                if q == 0:
                    s["UT"] = sb.tile([D, A], bf16, name=f"UT{b}", tag="UT",
                                      bufs=2)
                pu = ps.tile([128, 512], f32, name=f"pu{b}_{q}", tag="pu",
                             bufs=1)
                o = q * 512
                nc.tensor.matmul(pu[:], m_r[:], s["zT"][:, o:o + 512],
                                 start=True, stop=True)
                nc.vector.tensor_scalar(s["UT"][:, o:o + 512], pu[:],
                                        gw_col[:], None, ADD)

            def emit_scores(b, h, tk):
                s = st[b]
                lhs = s["zT"][:, tk * 128:(tk + 1) * 128]
                ps_t = ps.tile([128, 1024], f32, name=f"s{b}_{h}_{tk}",
                               tag="sc")
                for j in range(2):
                    o = h * 1024 + j * 512
                    nc.tensor.matmul(ps_t[:, j * 512:(j + 1) * 512],
                                     lhs, s["UT"][:, o:o + 512],
                                     start=True, stop=True)
                eT = sb.tile([128, 1024], bf16, name=f"e{b}_{h}_{tk}",
                             tag="eT", bufs=10)
                if tk in SCHR_TKS:
                    nc.vector.tensor_scalar(eT.bitcast(i16)[:], ps_t[:],
                                            A16S, B16S, MULT, ADD)
                else:
                    nc.scalar.activation(eT[:], ps_t[:], AF.Exp,
                                         scale=SCALE)
                return eT

            def emit_nd(b, h, tk, eT):
                s = st[b]
                wlt = s["wl"][:, 2 * tk:2 * tk + 2]
                for c in range(2):
                    nc.tensor.matmul(
                        s["nd"][c], wlt, eT[:, c * 512:(c + 1) * 512],
                        start=(tk == 0), stop=(tk == NT - 1))

            def alloc_nd(b, h):
                s = st[b]
                if ND1BANK:
                    ndt = ps.tile([34, 512], f32, name=f"nd{b}_{h}",
                                  tag="nd", bufs=1)
                    s["nd"] = [ndt[0:2, :], ndt[32:34, :]]
                else:
                    s["nd"] = [ps.tile([2, 512], f32, name=f"nd{b}_{h}_{c}",
                                       tag=f"nd{c}", bufs=1)[:]
                               for c in range(2)]

            def emit_nd_flush(b, h):
                s = st[b]
                for c in range(2):
                    o = h * 1024 + c * 512
                    nc.vector.tensor_copy(s["ndall"][0:2, o:o + 512],
                                          s["nd"][c])

            o16 = sb.tile([16, B_PER * 128], f32, name="o16")

            def emit_finale(b, step):
                s = st[b]
                if step == 0:
                    s["num16"] = sb.tile([16, 128], f32, name=f"num16{b}",
                                         tag="num16", bufs=2)
                    nc.sync.dma_start(
                        s["num16"][:, :],
                        s["ndall"][0:1, :].rearrange(
                            "one (t p) -> one t p", p=128))
                elif step == 1:
                    s["den16"] = sb.tile([16, 128], f32, name=f"den16{b}",
                                         tag="den16", bufs=2)
                    nc.sync.dma_start(
                        s["den16"][:, :],
                        s["ndall"][1:2, :].rearrange(
                            "one (t p) -> one t p", p=128))
                elif step == 2:
                    s["rcp"] = sb.tile([16, 128], f32, name=f"rcp{b}",
                                       tag="rcp", bufs=2)
                    nc.vector.reciprocal(s["rcp"][:], s["den16"][:])
                else:
                    nc.vector.tensor_tensor(
                        o16[:, b * 128:(b + 1) * 128], s["num16"][:],
                        s["rcp"][:], MULT)
                    st.pop(b)

            # ---- prologue: batch 0 setup ----
            emit_transpose(0, 0)
            emit_transpose(0, 1)
            emit_z_dma(0, 0)
            emit_z_dma(0, 1)
            nc.sync.dma_start(wvb[:], wvb_d[:])
            for q in range(4):
                emit_ut_quarter(0, q)
            emit_w_mult(0)
            emit_w_reduce(0)
            emit_wl(0)

            def setup_piece(b, nxt, h, tk):
                # finale of b-1 early in b; setup of b+1 spread through b
                if h == 0 and tk in (1, 2, 3, 4) and (b - 1) in st:
                    emit_finale(b - 1, tk - 1)
                if nxt is None:
                    return
                # batch 0 is short (no fill phase): run its successor's
                # setup ~6 ticks earlier so DVE finishes UT(1) in time
                step = h * NT + tk
                if step == 5:
                    emit_transpose(nxt, 0)
                elif step == 7:
                    emit_transpose(nxt, 1)
                elif step == 9:
                    emit_z_dma(nxt, 0)
                elif step == 11:
                    emit_z_dma(nxt, 1)
                elif step == 16:
                    emit_w_mult(nxt)
                elif step == 18:
                    emit_w_reduce(nxt)
                elif step == 20:
                    emit_wl(nxt)
                elif step in (22, 24, 26, 28):
                    emit_ut_quarter(nxt, (step - 22) // 2)

            def emit_row_finale(b, h, part):
                # fast 1-descriptor path for the exposed last-batch tail;
                # part 0 (DMA) emitted well before part 1 (compute) so the
                # DVE never head-of-line blocks on a fresh DMA
                s = st[b]
                o = h * 1024
                if part == 0:
                    s[f"denr{h}"] = sb.tile([1, 1024], f32,
                                            name=f"denr{b}_{h}",
                                            tag=f"denr{h}", bufs=1)
                    nc.sync.dma_start(s[f"denr{h}"][:],
                                      s["ndall"][1:2, o:o + 1024])
                    return
                rcp_row = sb.tile([1, 1024], f32, name=f"rcpr{b}_{h}",
                                  tag="rcpr", bufs=2)
                nc.vector.reciprocal(rcp_row[:], s[f"denr{h}"][:])
                nc.vector.tensor_tensor(orow3[0:1, o:o + 1024],
                                        s["ndall"][0:1, o:o + 1024],
                                        rcp_row[:], MULT)

            orow3 = sb.tile([1, A], f32, name="orow3")

            for b in range(B_PER):
                s = st[b]
                s["ndall"] = sb.tile([2, A], f32, name=f"ndall{b}",
                                     tag="ndall", bufs=2)
                nxt = b + 1 if b + 1 < B_PER else None
                for h in range(NH):
                    alloc_nd(b, h)
                    pend = []
                    for tk in range(NT):
                        eT = emit_scores(b, h, tk)
                        pend.append((tk, eT))
                        keep = 3 if tk < 13 else 15 - tk
                        while len(pend) > keep:
                            ptk, peT = pend.pop(0)
                            emit_nd(b, h, ptk, peT)
                        setup_piece(b, nxt, h, tk)
                    emit_nd_flush(b, h)
                if nxt is None:
                    for stp in range(4):
                        emit_finale(b, stp)

            # single combined output DMA
            nc.sync.dma_start(
                out_d[:, :].rearrange("b (t p) -> t b p", p=128),
                o16.rearrange("t (b p) -> t b p", p=128))

    nc.compile()
    return nc


def run(inputs: dict, trace: bool = False):
    _install_axon_shim()
    import ml_dtypes
    from concourse.bass_utils import run_bass_kernel_spmd

    z = np.asarray(inputs["z"], dtype=np.float32)
    Wq = np.asarray(inputs["Wq"], dtype=np.float64)
    bq = np.asarray(inputs["bq"], dtype=np.float64)
    Wk = np.asarray(inputs["Wk"], dtype=np.float64)
    Wv = np.asarray(inputs["Wv"], dtype=np.float64)
    bv = np.asarray(inputs["bv"], dtype=np.float64)
    Wo = np.asarray(inputs["Wo"], dtype=np.float64)
    bo = np.asarray(inputs["bo"], dtype=np.float64)

    # host-side weight algebra (tiny, exact in float64)
    m_lhs = (Wq.T @ Wk).astype(np.float32)            # [d, d']
    gw = (Wk.T @ bq).astype(np.float32).reshape(D, 1)
    wv = (Wv.T @ Wo[0]).astype(np.float32)            # [d]
    wvb = np.broadcast_to(np.tile(wv, A // D), (128, A)).astype(
        ml_dtypes.bfloat16)
    cbo_val = float(bv @ Wo[0] + bo[0])

    z_bf = z.astype(ml_dtypes.bfloat16)

    nc = _build_program(cbo_val)

    in_maps = []
    for c in range(N_CORES):
        in_maps.append({
            "z": z_bf[c * B_PER:(c + 1) * B_PER],
            "m_lhs": m_lhs,
            "gw": gw,
            "wvb": np.ascontiguousarray(wvb),
        })
    res = run_bass_kernel_spmd(nc, in_maps, core_ids=list(range(N_CORES)),
                               trace=trace)
    out = np.concatenate([res.results[c]["out"] for c in range(N_CORES)],
                         axis=0)
    return out.reshape(B, A, 1).astype(np.float32), res


def kernel(**inputs) -> np.ndarray:
    out, _ = run(inputs, trace=False)
    return out
